# revision 5
# baseline (speedup 1.0000x reference)
"""Band-sparse (local block) attention on 8 TRN2 NeuronCores.

Problem: q,k,v [4096, 8, 64] f32; block size 128; banded block mask with 4
blocks each side of the diagonal (window 512). pair_bias is unused.

Sharding: one head per NeuronCore (8 heads / 8 cores). Each core computes
its head's banded attention; host slices/transposes inputs and reassembles
the output.

Per-core algorithm (head h), v3:
  Layout: qt [128, 4096] bf16 = q^T pre-scaled by 16*log2(e), duplicated
          into partitions 64..127 (chunked into 4 tiles of 1024 cols so
          compute starts as soon as the first chunk lands); kt likewise;
          vo [128, 32, 65] in 4 chunk tiles = per key block j-major V
          plus a ones column.
  QK is row-tiled: even key blocks stream on PE array rows 0..63, odd
  blocks on rows 64..127 (d=64 contraction needs only half the array,
  so the two blocks of a pair run concurrently -> 2x QK throughput).
  Scores (pre-scaled: U = 128*log2 e^(s/8)) land in PSUM f32 [128, w<=1152].
  The exp of each block is split across BOTH engines, concurrently:
    - ScalarE ACT, cols [0, 512):   p = exp(U * ln2/128) -> bf16
    - VectorE custom op, cols [512, w): EXP_BITS_ANT emits the bf16 BIT
      PATTERN of ~exp(s/8) as int16 in one instruction (magic-number
      round trick + quadratic correction; centered log err ~0.55%, mean
      log err ~0 so the two engines' scales agree).
  Both write into one int16 tile that PV reads as bf16.
  PV accumulates o_ps[65, 512] per 4-block query group in PSUM over the
  12 intersecting key blocks (ones row = softmax denominator), delayed
  one pair behind QK so it never waits on a same-pair exp.
  Evacuate o_ps via ScalarE copy, DMA out as ot [65, 4096] f32.
Host: out = (ot[:64] / ot[64:65]).T per head.
"""

import os
import sys

import numpy as np


def _ensure_path():
    try:
        import concourse  # noqa: F401
    except ImportError:
        for p in ("/opt/trn_rl_repo", "/root/.axon_site/_ro/trn_rl_repo"):
            if os.path.isdir(p) and p not in sys.path:
                sys.path.insert(0, p)


_ensure_path()

import ml_dtypes  # noqa: E402

import concourse.bacc as bacc  # noqa: E402
import concourse.tile as tile  # noqa: E402
from concourse import mybir  # noqa: E402
from concourse.bass_utils import run_bass_kernel_spmd  # noqa: E402

N, H, D, B = 4096, 8, 64, 128
NROW = N // B  # 32 row/key blocks
BPS = 4  # band: blocks per side
F32 = mybir.dt.float32
BF16 = mybir.dt.bfloat16
I16 = mybir.dt.int16
NP_BF16 = ml_dtypes.bfloat16
MAXW = (2 * BPS + 1) * B  # 1152: widest band span
CHUNK = 1024  # input DMA chunk (columns)
NCH = N // CHUNK

# exp constants. The host pre-scales q by C0SCALE so the PSUM score is
# U = 128 * log2(exp(s/8)). ACT recovers exp(s/8) via its free affine
# scale; the DVE custom op consumes U directly.
C0SCALE = float(16 * np.log2(np.e))
ACT_SCALE = float(np.log(2.0) / 128.0)
CN = -70.1359130
CS = 0.00228512688
C1MAGIC = float(np.float32(1.5 * 2**30 + 16256))
CBX = 16244.620445507204


def _register_exp_bits():
    """Register the EXP_BITS_ANT custom DVE op (idempotent).

    out_i16 = round_to_int16( U + CS*(|U - K| + CN)^2 + CBX ),
    K = (U + C1MAGIC) - C1MAGIC = 128*round(U/128) exactly.
    """
    import concourse.dve_ops as dve_ops
    from concourse.dve_spec import (
        Spec, Src0, C0, C1, C2, C3, Bin, AluOp, lower, sq,
        _spill_c3_to_src1, _has_src1,
    )
    from concourse.dve_uop import DveOpSpec

    name = "EXP_BITS_ANT"
    for op in dve_ops.OPS:
        if op.name == name:
            return op

    t = Src0 + C1
    k3 = t - C1
    a = Bin(AluOp.ABSOLUTE_DIFF, Src0, k3)
    n1 = a + C0
    n2 = sq(n1)
    n3 = n2 * C2
    body = _spill_c3_to_src1((Src0 + n3) + C3)

    def _np_reference(in0, in1, s0, s1, imm2):
        f32 = np.float32
        U = in0.astype(f32)
        tt = f32(U + f32(s1))
        kk = f32(tt - f32(s1))
        aa = f32(np.abs(f32(U - kk)))
        nn1 = f32(aa + f32(s0))
        nn3 = f32(f32(nn1 * nn1) * f32(imm2))
        return f32(f32(U + nn3) + in1)

    spec = Spec(body=body, reference=_np_reference)
    opcode = dve_ops._CUSTOM_DVE_ROW_BASE + len(dve_ops.OPS)
    shas = {}
    for ver in ("v3", "v4"):
        s = DveOpSpec(
            name=name, opcode=opcode, uops=lower(spec, ver=ver),
            rd1_en=_has_src1(spec),
        )
        shas[ver] = s.sha(ver)
    op = dve_ops.DveOp(name, spec, subdim=False, uops_sha=shas)
    dve_ops._SUB_OPCODE_FOR_NAME[name] = opcode
    dve_ops.OPS.append(op)
    dve_ops.CUSTOM_DVE_SPECS[name] = spec
    return op


EXP_BITS = _register_exp_bits()


def _band(c):
    """Valid query-block range for key block c (inclusive)."""
    return max(0, c - BPS), min(NROW - 1, c + BPS)


def _pieces(q_lo, w):
    """Split band [q_lo, q_lo+w) into matmul pieces <=512 wide that do not
    cross input-chunk boundaries (1024 abs) nor the relative col-512/1024
    splits (PSUM banks / the ACT-DVE exp split)."""
    cuts = {512, 1024, w}
    o = CHUNK - (q_lo % CHUNK)
    while o < w:
        cuts.add(o)
        o += CHUNK
    out, prev = [], 0
    for c in sorted(cuts):
        if c > w:
            break
        while c - prev > 512:
            out.append((prev, prev + 512))
            prev += 512
        if c > prev:
            out.append((prev, c))
            prev = c
    return out


def _build_nc():
    nc = bacc.Bacc(None)
    qt_d = nc.dram_tensor("qt", [2 * D, N], BF16, kind="ExternalInput")
    kt_d = nc.dram_tensor("kt", [2 * D, N], BF16, kind="ExternalInput")
    vo_d = nc.dram_tensor("vo", [B, NROW, D + 1], BF16, kind="ExternalInput")
    ot_d = nc.dram_tensor("ot", [D + 1, N], F32, kind="ExternalOutput")

    with tile.TileContext(nc) as tc:
        with (
            tc.tile_pool(name="io", bufs=1) as io_pool,
            tc.tile_pool(name="pc", bufs=10) as pc_pool,
            tc.tile_pool(name="st", bufs=2, space="PSUM") as st_pool,
            tc.tile_pool(name="acc", bufs=2, space="PSUM") as acc_pool,
            tc.tile_pool(name="ev", bufs=2) as ev_pool,
        ):
            qt_t = [io_pool.tile([2 * D, CHUNK], BF16, name=f"qt{i}")
                    for i in range(NCH)]
            kt_t = [io_pool.tile([2 * D, CHUNK], BF16, name=f"kt{i}")
                    for i in range(NCH)]
            vo_t = [io_pool.tile([B, NROW // NCH, D + 1], BF16,
                                 name=f"vo{i}") for i in range(NCH)]
            cbx = io_pool.tile([B, 1], F32)
            wz = io_pool.tile([B, 512], BF16)

            def dma_in(eng, which, i):
                if which == "vo":
                    nb = NROW // NCH
                    eng.dma_start(out=vo_t[i][:, :, :],
                                  in_=vo_d[:, i * nb:(i + 1) * nb, :])
                else:
                    src = kt_d if which == "kt" else qt_d
                    dst = kt_t[i] if which == "kt" else qt_t[i]
                    eng.dma_start(out=dst[:, :],
                                  in_=src[:, i * CHUNK:(i + 1) * CHUNK])

            # Input DMAs first. qt0 issues from the (initially idle)
            # scalar queue so the first chunks land ~1.5us in; kt/qt
            # alternate on sync; vo + the rest go on gpsimd. One DMA per chunk-tile so
            # the (tile-granular) dependencies release incrementally.
            nc.vector.memset(cbx, CBX)
            dma_in(nc.scalar, "qt", 0)
            for which, i in [("kt", 0), ("qt", 1), ("kt", 2), ("qt", 3)]:
                dma_in(nc.sync, which, i)
            nc.gpsimd.memset(wz, 0.0)
            for which, i in [("vo", 0), ("kt", 1), ("vo", 1), ("qt", 2),
                             ("kt", 3), ("vo", 2), ("vo", 3)]:
                dma_in(nc.gpsimd, which, i)

            # HAM warmup on array rows 0..63 only (stream B can overlap):
            # covers the PE from ~0.9us until the first QK pair (~2.1us).
            wps = acc_pool.tile([B, 512], F32, name="wps", tag="ops")
            for _ in range(4):
                nc.tensor.matmul(wps[:, :], wz[0:D, :B], wz[0:D, :],
                                 start=True, stop=True)

            P = {}  # c -> (int16 tile, q_lo, w)
            o_ps = {}

            def qk(c):
                r_lo, r_hi = _band(c)
                q_lo = r_lo * B
                w = (r_hi - r_lo + 1) * B
                half = slice(0, D) if c % 2 == 0 else slice(D, 2 * D)
                st = st_pool.tile([B, MAXW], F32, tag="st")
                mms = []
                for o, e in _pieces(q_lo, w):
                    ch = (q_lo + o) // CHUNK
                    co = (q_lo + o) % CHUNK
                    mms.append((
                        st[:, o:e],
                        kt_t[c // (NROW // NCH)][
                            half, (c % (NROW // NCH)) * B:
                            (c % (NROW // NCH)) * B + B],
                        qt_t[ch][half, co:co + (e - o)],
                    ))
                return st, q_lo, w, mms

            def emit_exp(c, st, q_lo, w):
                pc = pc_pool.tile([B, MAXW], I16, tag="pc")
                nc.scalar.activation(
                    pc[:, 0:512].bitcast(BF16), st[:, 0:512],
                    mybir.ActivationFunctionType.Exp, scale=ACT_SCALE,
                )
                nc.vector._custom_dve(
                    EXP_BITS, out=pc[:, 512:w], in0=st[:, 512:w],
                    in1=cbx[:, 0:1], s0=CN, s1=C1MAGIC, imm2=CS,
                )
                P[c] = (pc, q_lo, w)

            def qk_pair(c0, c1):
                st0, ql0, w0, mm0 = qk(c0)
                st1, ql1, w1, mm1 = qk(c1)
                for i in range(max(len(mm0), len(mm1))):
                    if i < len(mm0):
                        nc.tensor.matmul(*mm0[i], start=True, stop=True)
                    if i < len(mm1):
                        nc.tensor.matmul(*mm1[i], start=True, stop=True)
                emit_exp(c0, st0, ql0, w0)
                emit_exp(c1, st1, ql1, w1)

            def pv(g, c, first_call, last_call):
                # accumulate key block c's contribution to query group g.
                # start=True once per accumulator bank (first matmul),
                # stop=True on the very last. Rows split into runs by "is
                # this row's first contribution" so each matmul's bytes
                # are uniformly fresh or accumulating.
                r_lo = max(4 * g, c - BPS, 0)
                r_hi = min(4 * g + 3, c + BPS, NROW - 1)
                if r_lo > r_hi:
                    return
                pc, q_lo, _w = P[c]
                runs = []
                for r in range(r_lo, r_hi + 1):
                    fresh = c == max(0, r - BPS)
                    if runs and runs[-1][2] == fresh:
                        runs[-1][1] = r
                    else:
                        runs.append([r, r, fresh])
                nb = NROW // NCH
                for i, (ra, rb, _fresh) in enumerate(runs):
                    nc.tensor.matmul(
                        o_ps[g][:, (ra - 4 * g) * B:(rb + 1 - 4 * g) * B],
                        vo_t[c // nb][:, c % nb, :],
                        pc[:, ra * B - q_lo:(rb + 1) * B - q_lo].bitcast(
                            BF16),
                        start=first_call and i == 0,
                        stop=last_call and i == len(runs) - 1,
                    )

            def evac(g):
                ev = ev_pool.tile([D + 1, 4 * B], F32, tag="ev")
                nc.scalar.copy(ev[:, :], o_ps[g][:, :])
                nc.sync.dma_start(
                    out=ot_d[:, 4 * g * B:(4 * g + 4) * B], in_=ev[:, :]
                )

            def pv_step(step):
                # consume block step-1 into the groups that need it.
                for g in range(NROW // 4):
                    s0 = 4 * g + 1
                    c_first = max(0, 4 * g - BPS)
                    c_last = min(NROW - 1, 4 * g + BPS + 3)
                    if step == s0:
                        o_ps[g] = acc_pool.tile(
                            [D + 1, 4 * B], F32, name="ops", tag="ops"
                        )
                        for cc in range(c_first, s0):
                            pv(g, cc, cc == c_first, cc == c_last)
                    elif s0 < step <= 4 * g + BPS + 4:
                        c = step - 1
                        pv(g, c, c == c_first, c == c_last)
                    if step == c_last + 1:
                        evac(g)

            # PV runs one full pair behind QK so it never waits on a
            # same-pair exp.
            for pair in range(NROW // 2 + 1):
                if pair < NROW // 2:
                    qk_pair(2 * pair, 2 * pair + 1)
                for s in (2 * pair - 1, 2 * pair):
                    if 1 <= s <= NROW:
                        pv_step(s)

    nc.compile()
    return nc


_NC = None


def _get_nc():
    global _NC
    if _NC is None:
        _NC = _build_nc()
    return _NC


def _make_in_maps(q, k, v):
    q = np.ascontiguousarray(q, dtype=np.float32)
    k = np.ascontiguousarray(k, dtype=np.float32)
    v = np.ascontiguousarray(v, dtype=np.float32)
    in_maps = []
    for h in range(H):
        qT = (q[:, h, :].T * np.float32(C0SCALE)).astype(NP_BF16)  # [64, N]
        kT = k[:, h, :].T.astype(NP_BF16)
        qT2 = np.ascontiguousarray(np.concatenate([qT, qT], axis=0))
        kT2 = np.ascontiguousarray(np.concatenate([kT, kT], axis=0))
        vb = v[:, h, :].reshape(NROW, B, D).transpose(1, 0, 2)  # [128, 32, 64]
        vo = np.concatenate(
            [vb, np.ones((B, NROW, 1), np.float32)], axis=2
        ).astype(NP_BF16)  # [128, 32, 65]
        in_maps.append(
            {"qt": qT2, "kt": kT2, "vo": np.ascontiguousarray(vo)}
        )
    return in_maps


def run(q, k, v, trace=False, **trace_kwargs):
    """Returns (out [4096, 8, 64] f32, BassKernelResults)."""
    nc = _get_nc()
    in_maps = _make_in_maps(q, k, v)
    res = run_bass_kernel_spmd(
        nc, in_maps, list(range(H)), trace=trace, **trace_kwargs
    )
    out = np.empty((N, H, D), dtype=np.float32)
    for h in range(H):
        ot = res.results[h]["ot"]  # [65, 4096]
        out[:, h, :] = (ot[:D] / ot[D:D + 1]).T
    return out, res


def kernel(q, k, v, pair_bias=None):
    out, _ = run(q, k, v)
    return out
